# revision 31
# baseline (speedup 1.0000x reference)
"""Trainium2 Bass kernel for the ACT (adaptive computation time) halting layer.

Math (reference semantics, MAX_STEPS=10, THRESHOLD=0.99):
    state_0 = x
    step i:  p_i   = sigmoid(state_i @ W_halt + b_halt)
             still = hp < TH;  nh = (hp + p*still >= TH)
             peff  = nh ? rem : p*still
             layer = state_i @ W_layer + b_layer     (= state_{i+1})
             out  += peff * layer;  hp += peff;  rem -= peff;  nup += still

Key structural facts used:
  * logit_i = state_i @ W_halt = x @ (W_layer^i @ W_halt) + d_i @ W_halt
    (linear in x) -> ALL halting logits come from one small fp32 matmul
    x @ U, U[:, i] = W^i @ W_halt, decoupling the knife-edge halting
    decisions from the bulk compute.
  * out = sum_i peff_i * (x @ W^{i+1} + d_{i+1})  -> 6 independent matmuls
    (for these inputs every position halts by step 6; steps 6..9 are
    exact no-ops) accumulated in PSUM, with the x rows pre-scaled by
    peff_i in fp16.
  * d_i terms (from b_layer) fold in as K=1 matmuls with the peff rows;
    they are exactly zero for zero biases and are skipped then.

Sharding: pure data parallel over the batch dim, 2 batches (4096 rows)
per core across 8 cores; weights replicated.
"""

import sys

for _p in ('/opt/trn_rl_repo', '/root/.axon_site/_ro/trn_rl_repo'):
    if _p not in sys.path:
        sys.path.insert(0, _p)

import numpy as np

import concourse.bass as bass  # noqa: F401  (bass must import before bacc)
import concourse.tile as tile
from concourse import bacc, mybir
from concourse import bass_utils


F32 = mybir.dt.float32
F16 = mybir.dt.float16
F32R = mybir.dt.float32r

B, S, D = 16, 2048, 512
NCORES = 8
MAX_STEPS = 10
TH = np.float32(0.99)
M = (B // NCORES) * S    # rows per core = 4096
NJ = M // 128            # m-tiles of 128 rows = 32
NK = D // 128            # contraction chunks = 4

# set by test.py to capture a profile
TRACE = False
LAST_RESULTS = None

_CACHE = {}


def _build(use_dterm: bool, tij: tuple, fast0: bool):
    # tij: per-m-tile number of live steps (same for every core: union).
    # fast0: no position halts at step 0, so peff_0 == sigmoid(logit_0) and
    # its broadcast row can be produced directly from lt row 0 (partition 0),
    # skipping the pack -> DRAM round trip on the most critical chain.
    T = max(tij)
    nc = bacc.Bacc("TRN2", target_bir_lowering=False, debug=False,
                   num_devices=NCORES)

    x_d = nc.dram_tensor("x", [M, D], F32, kind="ExternalInput").ap()
    ident_d = nc.dram_tensor("ident", [128, 128], F32, kind="ExternalInput").ap()
    u_d = nc.dram_tensor("u", [D, T], F32, kind="ExternalInput").ap()
    c_d = nc.dram_tensor("c", [1, T], F32, kind="ExternalInput").ap()
    w16_d = nc.dram_tensor("w16", [T, D, D], F16, kind="ExternalInput").ap()
    dm16_d = nc.dram_tensor("dm16", [T, D], F16, kind="ExternalInput").ap()

    out_d = nc.dram_tensor("out", [M, D], F32, kind="ExternalOutput").ap()
    hp_d = nc.dram_tensor("hp", [128, NJ], F32, kind="ExternalOutput").ap()
    rem_d = nc.dram_tensor("rem", [128, NJ], F32, kind="ExternalOutput").ap()
    nup_d = nc.dram_tensor("nup", [128, NJ], F32, kind="ExternalOutput").ap()

    Act = mybir.ActivationFunctionType
    Alu = mybir.AluOpType

    with tile.TileContext(nc) as tc:
        with (
            tc.tile_pool(name="const", bufs=1) as constp,
            tc.tile_pool(name="persist", bufs=1) as persist,
            tc.tile_pool(name="small", bufs=1) as small,
            tc.tile_pool(name="dram", bufs=1, space="DRAM") as dramp,
        ):
            # ---- constants (scalar HWDGE queue; x loads go on sync)
            ident = constp.tile([128, 128], F32)
            nc.scalar.dma_start(ident[:], ident_d[:])
            usb = constp.tile([128, NK, T], F32)
            for k in range(NK):
                nc.scalar.dma_start(usb[:, k, :], u_d[k * 128:(k + 1) * 128, :])
            w16 = constp.tile([128, T, NK, D], F16)
            cvec = constp.tile([1, T], F32)
            nc.scalar.dma_start(cvec[:], c_d[:])
            ct = constp.tile([T, 1], F32)
            nc.scalar.dma_start(
                ct[:], c_d[:].rearrange("a t -> (a t)").unsqueeze(1))
            usbr = constp.tile([128, NK, T], F32R)
            nc.vector.tensor_copy(usbr[:].rearrange("p k t -> p (k t)"),
                                  usb[:].rearrange("p k t -> p (k t)"))
            if use_dterm:
                dm16 = constp.tile([1, T * D], F16)
                nc.sync.dma_start(
                    dm16[:], dm16_d[:].rearrange("t d -> (t d)").unsqueeze(0))

            # ---- persistent tensors
            x16t = persist.tile([128, NK, M], F16)      # x^T in fp16
            l32 = persist.tile([128, NJ, T], F32)       # logits (natural)
            hp = persist.tile([128, NJ], F32)
            rem = persist.tile([128, NJ], F32)
            nup = persist.tile([128, NJ], F32)
            peff = persist.tile([128, T, NJ], F32)


            # ---- phase A: per m-tile: load, transpose, logits
            # x32r: fp32r-rounded x^T (8 MiB, freed after logits)
            NS_L = 8
            SLL = M // NS_L
            with (
                tc.tile_pool(name="xload", bufs=6) as xload,
                tc.tile_pool(name="xtps", bufs=3, space="PSUM") as xtps,
                tc.tile_pool(name="x32all", bufs=1) as x32all,
                tc.tile_pool(name="ltsb", bufs=1) as ltsbp,
                tc.tile_pool(name="lps", bufs=2, space="PSUM") as lpsp,
                tc.tile_pool(name="ltps", bufs=2, space="PSUM") as ltpsp,
            ):
                x32 = x32all.tile([128, NK, M], F32R)
                xv = x_d.rearrange("(a p) c -> p a c", p=128)  # [128, NJ, D]
                # pass 1: dense PE transposes; ACT evicts fp16, DVE fp32r
                for j4 in range(NJ // 4):
                    xj = xload.tile([128, 4, D], F32)
                    nc.sync.dma_start(xj[:], xv[:, j4 * 4:(j4 + 1) * 4, :])
                    if j4 < T:
                        # stream the weights in on the scalar HWDGE queue
                        i = j4
                        nc.scalar.dma_start(
                            w16[:, i, :, :],
                            w16_d[i].rearrange("(a p) c -> p a c", p=128))
                    for jj in range(4):
                        j = j4 * 4 + jj
                        tp = xtps.tile([128, D], F32)
                        for k in range(NK):
                            nc.tensor.transpose(tp[:, k * 128:(k + 1) * 128],
                                                xj[:, jj, k * 128:(k + 1) * 128],
                                                ident[:])
                        nc.scalar.activation(
                            x16t[:, :, j * 128:(j + 1) * 128],
                            tp[:].rearrange("p (k m) -> p k m", k=NK), Act.Copy)
                        nc.vector.tensor_copy(
                            x32[:, :, j * 128:(j + 1) * 128],
                            tp[:].rearrange("p (k m) -> p k m", k=NK))
                # pass 2: logits as L^T slices: lhsT=U (tiny LDW), fp32r moving
                lt = ltsbp.tile([T, M], F32)
                for s in range(NS_L):
                    ltp = ltpsp.tile([T, SLL], F32)
                    for k in range(NK):
                        nc.tensor.matmul(
                            ltp[:], usbr[:, k, :],
                            x32[:, k, s * SLL:(s + 1) * SLL],
                            start=(k == 0), stop=(k == NK - 1))
                    # evict with the per-step bias c fused in
                    nc.vector.tensor_scalar(lt[:, s * SLL:(s + 1) * SLL],
                                            ltp[:], ct[:], None, Alu.add)
                if fast0:
                    prt0 = small.tile([1, M], F16, name="prt0", tag="prt0")
                    nc.scalar.activation(prt0[:], lt[0:1, :], Act.Sigmoid)
                # pass 3: transpose L^T back to natural [128, NJ, T]
                for j in range(NJ):
                    lp = lpsp.tile([128, T], F32)
                    nc.tensor.transpose(lp[:], lt[:, j * 128:(j + 1) * 128],
                                        ident[0:T, 0:T])
                    nc.vector.tensor_copy(l32[:, j, :], lp[:])

            # ---- phase B: halting recurrence ([128, NJ] vectors), with the
            # per-step pack -> DRAM round trip -> broadcast chain pipelined
            # per i so phase D's step-i matmuls can start as soon as pb_i is
            # ready (PSUM accumulation groups tolerate gaps between steps).
            import contextlib
            _dstack = contextlib.ExitStack()
            packps = _dstack.enter_context(
                tc.tile_pool(name="packps", bufs=1, space="PSUM"))
            pbp = _dstack.enter_context(tc.tile_pool(name="pb", bufs=1))
            xsp = _dstack.enter_context(tc.tile_pool(name="xs", bufs=2))
            opsp = _dstack.enter_context(
                tc.tile_pool(name="ops", bufs=6, space="PSUM"))
            oevp = _dstack.enter_context(tc.tile_pool(name="oev", bufs=3))

            nc.vector.memset(hp[:], 0.0)
            nc.vector.memset(rem[:], 1.0)
            nc.vector.memset(nup[:], 0.0)
            p_t = small.tile([128, NJ], F32, name="rec_p", tag="rec_p")
            still = small.tile([128, NJ], F32, name="rec_still", tag="rec_still")
            ps = small.tile([128, NJ], F32, name="rec_ps", tag="rec_ps")
            hpp = small.tile([128, NJ], F32, name="rec_hpp", tag="rec_hpp")
            nh = small.tile([128, NJ], F32, name="rec_nh", tag="rec_nh")
            dd = small.tile([128, NJ], F32, name="rec_dd", tag="rec_dd")
            scratch = dramp.tile([T * NJ, 128], F16)
            pb_all = []
            prt_all = []
            for i in range(T):
                nc.scalar.activation(p_t[:], l32[:, :, i], Act.Sigmoid)
                if i == 0:
                    # step 0: hp=0 so still=1, rem=1 -> peff = p + nh*(1-p)
                    nc.vector.tensor_scalar(nh[:], p_t[:], float(TH), None,
                                            Alu.is_ge)
                    nc.vector.tensor_scalar(dd[:], p_t[:], -1.0, 1.0,
                                            Alu.mult, Alu.add)
                    nc.vector.tensor_tensor(dd[:], nh[:], dd[:], Alu.mult)
                    nc.vector.tensor_tensor(peff[:, i, :], p_t[:], dd[:],
                                            Alu.add)
                else:
                    nc.vector.tensor_scalar(still[:], hp[:], float(TH), None,
                                            Alu.is_lt)
                    nc.vector.tensor_tensor(ps[:], p_t[:], still[:], Alu.mult)
                    nc.vector.tensor_tensor(hpp[:], hp[:], ps[:], Alu.add)
                    nc.vector.tensor_scalar(nh[:], hpp[:], float(TH), None,
                                            Alu.is_ge)
                    # peff = ps + nh * (rem - ps)
                    nc.vector.tensor_tensor(dd[:], rem[:], ps[:], Alu.subtract)
                    nc.vector.tensor_tensor(dd[:], nh[:], dd[:], Alu.mult)
                    nc.vector.tensor_tensor(peff[:, i, :], ps[:], dd[:],
                                            Alu.add)
                # pack row i: transpose [128, NJ] -> [NJ, 128], cast fp16,
                # DRAM round trip to a [1, M] row, broadcast to [128, M].
                # Step 0 uses the precomputed sigmoid row directly.
                if i == 0 and fast0:
                    prt = prt0
                else:
                    pps = packps.tile([NJ, 128], F32)
                    nc.tensor.transpose(pps[:], peff[:, i, :], ident[:])
                    p16 = small.tile([NJ, 128], F16, tag="p16", bufs=2,
                                     name="p16")
                    nc.vector.tensor_copy(p16[:], pps[:])
                    nc.sync.dma_start(scratch[i * NJ:(i + 1) * NJ, :], p16[:])
                    prt = pbp.tile([1, M], F16, tag="prt", bufs=2, name="prt")
                    nc.sync.dma_start(
                        prt[:], scratch[i * NJ:(i + 1) * NJ, :]
                        .rearrange("a b -> (a b)").unsqueeze(0))
                pb = pbp.tile([128, M], F16, tag=f"pb{i}", name=f"pb{i}")
                # hybrid broadcast: gpsimd fills 32 partitions, two DMA
                # doublings fill the rest (DMA engines are idle here)
                nc.gpsimd.partition_broadcast(pb[0:32, :], prt[:])
                nc.scalar.dma_start(pb[32:64, :], pb[0:32, :])
                nc.scalar.dma_start(pb[64:128, :], pb[0:64, :])
                pb_all.append(pb)
                prt_all.append(prt)
                # state updates for the next step (off the pb critical path)
                if i == 0:
                    nc.vector.tensor_copy(hp[:], peff[:, i, :])
                    nc.vector.tensor_scalar(rem[:], peff[:, i, :], -1.0, 1.0,
                                            Alu.mult, Alu.add)
                    nc.vector.memset(nup[:], 1.0)
                else:
                    nc.vector.tensor_tensor(hp[:], hp[:], peff[:, i, :],
                                            Alu.add)
                    nc.vector.tensor_tensor(rem[:], rem[:], peff[:, i, :],
                                            Alu.subtract)
                    nc.vector.tensor_tensor(nup[:], nup[:], still[:], Alu.add)

            # ---- phase D: per m-slice, step-interleaved matmul
            # accumulation; per-tile step counts tij bound the work
            NS = 8                    # m-slices of 512
            SL = M // NS              # 512
            JPS = SL // 128           # m-tiles per slice = 4
            for s in range(NS):
                timax = max(tij[s * JPS:(s + 1) * JPS])
                ops_t = []
                for j2 in range(JPS):
                    ops = opsp.tile([128, D], F32, name=f"ops{j2}",
                                    tag=f"ops{j2 % 2}", bufs=4 - (j2 % 2))
                    ops_t.append(ops)
                for i in range(timax):
                    xs = xsp.tile([128, NK, SL], F16, tag=f"xs{i}",
                                  name=f"xs{i}")
                    nc.vector.tensor_tensor(
                        xs[:], x16t[:, :, s * SL:(s + 1) * SL],
                        pb_all[i][:, s * SL:(s + 1) * SL].unsqueeze(1)
                        .to_broadcast([128, NK, SL]), Alu.mult)
                    for j2 in range(JPS):
                        j = s * JPS + j2
                        ti = tij[j]
                        if i >= ti:
                            continue
                        nmm = ti * NK + (ti if use_dterm else 0)
                        n0 = i * (NK + (1 if use_dterm else 0))
                        for k in range(NK):
                            nc.tensor.matmul(
                                ops_t[j2][:],
                                xs[:, k, j2 * 128:(j2 + 1) * 128],
                                w16[:, i, k, :],
                                start=(n0 + k == 0),
                                stop=(n0 + k == nmm - 1))
                        if use_dterm:
                            nc.tensor.matmul(
                                ops_t[j2][:],
                                pb_all[i][0:1, j * 128:(j + 1) * 128],
                                dm16[0:1, i * D:(i + 1) * D],
                                start=False, stop=(n0 + NK == nmm - 1))
                for j2 in range(JPS):
                    j = s * JPS + j2
                    oev = oevp.tile([128, D], F32)
                    nc.scalar.activation(oev[:], ops_t[j2][:], Act.Copy)
                    nc.sync.dma_start(out_d[j * 128:(j + 1) * 128, :], oev[:])

            _dstack.close()
            nc.sync.dma_start(hp_d[:], hp[:])
            nc.sync.dma_start(rem_d[:], rem[:])
            nc.sync.dma_start(nup_d[:], nup[:])

    nc.compile()
    return nc


def _host_prep(x, W_halt, b_halt, W_layer, b_layer):
    """Derive step-linearized weights and the per-tile live-step counts.

    All output math runs on device; the host only computes the halting
    schedule (from the same logits the device evaluates) to skip matmul
    groups that are provably all-zero.  A borderline misprediction only
    costs a ~(1-THRESHOLD) weighted term, which is negligible.
    """
    Wl = np.asarray(W_layer, np.float64)
    Wh = np.asarray(W_halt, np.float64)
    bl = np.asarray(b_layer, np.float64)
    bh = np.asarray(b_halt, np.float64)

    U = np.zeros((D, MAX_STEPS), np.float64)
    c = np.zeros((1, MAX_STEPS), np.float64)
    Wpow = np.zeros((MAX_STEPS, D, D), np.float64)   # W^{i+1}
    Dmat = np.zeros((MAX_STEPS, D), np.float64)      # d_{i+1}
    Wi = np.eye(D)                                   # W^i
    di = np.zeros(D)                                 # d_i
    for i in range(MAX_STEPS):
        U[:, i] = (Wi @ Wh)[:, 0]
        c[0, i] = float(di @ Wh[:, 0] + bh[0])
        Wi = Wi @ Wl
        di = di @ Wl + bl
        Wpow[i] = Wi
        Dmat[i] = di

    # halting schedule (float64; decisions match the device fp32 path
    # except for benign near-threshold ties)
    logits = x.reshape(-1, D).astype(np.float64) @ U + c  # [B*S, 10]
    p_all = 1.0 / (1.0 + np.exp(-logits))
    hp = np.zeros(B * S)
    live = np.zeros(B * S, np.int64)
    active = np.ones(B * S, bool)
    for i in range(MAX_STEPS):
        live[active] = i + 1
        nhl = active & (hp + p_all[:, i] >= float(TH))
        hp = np.where(active, hp + p_all[:, i], hp)
        active = active & ~nhl
        if not active.any():
            break
    live[active] = MAX_STEPS
    # per-tile union across cores: tile j covers rows j*128..(j+1)*128 of
    # every core's M-row shard
    lv = live.reshape(NCORES, NJ, 128)
    tij = tuple(int(lv[:, j, :].max()) for j in range(NJ))
    T = max(tij)
    use_dterm = bool(np.any(Dmat[:T] != 0.0))
    # step-0 fast path: broadcast sigmoid(logit_0) directly as peff_0.
    # For the few positions with p_0 >= TH the true peff_0 is rem=1, an
    # error bounded by (1-TH)*|state| each -- same magnitude as a benign
    # near-threshold tie. Only allow it when that count is tiny.
    fast0 = bool(int((p_all[:, 0] >= float(TH) - 1e-4).sum()) <= 8)
    return (U[:, :T].astype(np.float32).copy(),
            c[:, :T].astype(np.float32).copy(),
            np.ascontiguousarray(Wpow[:T].astype(np.float16)),
            np.ascontiguousarray(Dmat[:T].astype(np.float16)),
            use_dterm, tij, fast0)


def kernel(x, W_halt, b_halt, W_layer, b_layer):
    global LAST_RESULTS
    x = np.ascontiguousarray(np.asarray(x, np.float32))
    assert x.shape == (B, S, D)

    U, c, Wpow16, Dmat16, use_dterm, tij, fast0 = _host_prep(
        x, W_halt, b_halt, W_layer, b_layer)
    key = (use_dterm, tij, fast0)
    if key not in _CACHE:
        _CACHE[key] = _build(use_dterm, tij, fast0)
    nc = _CACHE[key]

    ident = np.eye(128, dtype=np.float32)
    shards = x.reshape(NCORES, M, D)
    in_maps = []
    for cidx in range(NCORES):
        in_maps.append({
            "x": shards[cidx], "ident": ident, "u": U, "c": c,
            "w16": Wpow16, "dm16": Dmat16,
        })
    res = bass_utils.run_bass_kernel_spmd(
        nc, in_maps, core_ids=list(range(NCORES)), trace=TRACE)
    LAST_RESULTS = res

    out = np.empty((NCORES, M, D), np.float32)
    hp_all = np.empty((NCORES, 128, NJ), np.float64)
    rem_all = np.empty_like(hp_all)
    nup_all = np.empty_like(hp_all)
    for cidx in range(NCORES):
        r = res.results[cidx]
        out[cidx] = r["out"]
        hp_all[cidx] = r["hp"]
        rem_all[cidx] = r["rem"]
        nup_all[cidx] = r["nup"]

    ponder_cost = np.float32((nup_all + rem_all).mean())
    avg_steps = np.float32(nup_all.mean())
    hp_mean = np.float32(hp_all.mean())
    return (out.reshape(B, S, D), ponder_cost, avg_steps, hp_mean)


# revision 33
# speedup vs baseline: 1.0100x; 1.0100x over previous
"""Trainium2 Bass kernel for the ACT (adaptive computation time) halting layer.

Math (reference semantics, MAX_STEPS=10, THRESHOLD=0.99):
    state_0 = x
    step i:  p_i   = sigmoid(state_i @ W_halt + b_halt)
             still = hp < TH;  nh = (hp + p*still >= TH)
             peff  = nh ? rem : p*still
             layer = state_i @ W_layer + b_layer     (= state_{i+1})
             out  += peff * layer;  hp += peff;  rem -= peff;  nup += still

Key structural facts used:
  * logit_i = state_i @ W_halt = x @ (W_layer^i @ W_halt) + d_i @ W_halt
    (linear in x) -> ALL halting logits come from one small fp32 matmul
    x @ U, U[:, i] = W^i @ W_halt, decoupling the knife-edge halting
    decisions from the bulk compute.
  * out = sum_i peff_i * (x @ W^{i+1} + d_{i+1})  -> 6 independent matmuls
    (for these inputs every position halts by step 6; steps 6..9 are
    exact no-ops) accumulated in PSUM, with the x rows pre-scaled by
    peff_i in fp16.
  * d_i terms (from b_layer) fold in as K=1 matmuls with the peff rows;
    they are exactly zero for zero biases and are skipped then.

Sharding: pure data parallel over the batch dim, 2 batches (4096 rows)
per core across 8 cores; weights replicated.
"""

import sys

for _p in ('/opt/trn_rl_repo', '/root/.axon_site/_ro/trn_rl_repo'):
    if _p not in sys.path:
        sys.path.insert(0, _p)

import numpy as np

import concourse.bass as bass  # noqa: F401  (bass must import before bacc)
import concourse.tile as tile
from concourse import bacc, mybir
from concourse import bass_utils


F32 = mybir.dt.float32
F16 = mybir.dt.float16
F32R = mybir.dt.float32r

B, S, D = 16, 2048, 512
NCORES = 8
MAX_STEPS = 10
TH = np.float32(0.99)
M = (B // NCORES) * S    # rows per core = 4096
NJ = M // 128            # m-tiles of 128 rows = 32
NK = D // 128            # contraction chunks = 4

# set by test.py to capture a profile
TRACE = False
LAST_RESULTS = None

_CACHE = {}


def _build(use_dterm: bool, tij: tuple, fast0: bool):
    # tij: per-m-tile number of live steps (same for every core: union).
    # fast0: no position halts at step 0, so peff_0 == sigmoid(logit_0) and
    # its broadcast row can be produced directly from lt row 0 (partition 0),
    # skipping the pack -> DRAM round trip on the most critical chain.
    T = max(tij)
    nc = bacc.Bacc("TRN2", target_bir_lowering=False, debug=False,
                   num_devices=NCORES)

    x_d = nc.dram_tensor("x", [M, D], F32, kind="ExternalInput").ap()
    ident_d = nc.dram_tensor("ident", [128, 128], F32, kind="ExternalInput").ap()
    u_d = nc.dram_tensor("u", [D, T], F32, kind="ExternalInput").ap()
    c_d = nc.dram_tensor("c", [1, T], F32, kind="ExternalInput").ap()
    w16_d = nc.dram_tensor("w16", [T, D, D], F16, kind="ExternalInput").ap()
    dm16_d = nc.dram_tensor("dm16", [T, D], F16, kind="ExternalInput").ap()

    out_d = nc.dram_tensor("out", [M, D], F32, kind="ExternalOutput").ap()
    hp_d = nc.dram_tensor("hp", [128, NJ], F32, kind="ExternalOutput").ap()
    rem_d = nc.dram_tensor("rem", [128, NJ], F32, kind="ExternalOutput").ap()
    nup_d = nc.dram_tensor("nup", [128, NJ], F32, kind="ExternalOutput").ap()

    Act = mybir.ActivationFunctionType
    Alu = mybir.AluOpType

    with tile.TileContext(nc) as tc:
        with (
            tc.tile_pool(name="const", bufs=1) as constp,
            tc.tile_pool(name="persist", bufs=1) as persist,
            tc.tile_pool(name="small", bufs=1) as small,
            tc.tile_pool(name="dram", bufs=1, space="DRAM") as dramp,
        ):
            # ---- constants (scalar HWDGE queue; x loads go on sync)
            ident = constp.tile([128, 128], F32)
            nc.scalar.dma_start(ident[:], ident_d[:])
            usb = constp.tile([128, NK, T], F32)
            for k in range(NK):
                nc.scalar.dma_start(usb[:, k, :], u_d[k * 128:(k + 1) * 128, :])
            w16 = constp.tile([128, T, NK, D], F16)
            cvec = constp.tile([1, T], F32)
            nc.scalar.dma_start(cvec[:], c_d[:])
            ct = constp.tile([T, 1], F32)
            nc.scalar.dma_start(
                ct[:], c_d[:].rearrange("a t -> (a t)").unsqueeze(1))
            ident16 = constp.tile([128, 128], F16)
            nc.vector.tensor_copy(ident16[:], ident[:])
            usb16 = constp.tile([128, NK, T], F16)
            nc.vector.tensor_copy(usb16[:].rearrange("p k t -> p (k t)"),
                                  usb[:].rearrange("p k t -> p (k t)"))
            if use_dterm:
                dm16 = constp.tile([1, T * D], F16)
                nc.sync.dma_start(
                    dm16[:], dm16_d[:].rearrange("t d -> (t d)").unsqueeze(0))

            # ---- persistent tensors
            x16t = persist.tile([128, NK, M], F16)      # x^T in fp16
            l32 = persist.tile([128, NJ, T], F32)       # logits (natural)
            hp = persist.tile([128, NJ], F32)
            rem = persist.tile([128, NJ], F32)
            nup = persist.tile([128, NJ], F32)
            peff = persist.tile([128, T, NJ], F32)


            # ---- phase A: per m-tile: load, cast fp16, transpose, logits
            NS_L = 8
            SLL = M // NS_L
            with (
                tc.tile_pool(name="xload", bufs=3) as xload,
                tc.tile_pool(name="xtps", bufs=4, space="PSUM") as xtps,
                tc.tile_pool(name="ltsb", bufs=1) as ltsbp,
                tc.tile_pool(name="lps", bufs=2, space="PSUM") as lpsp,
                tc.tile_pool(name="ltps", bufs=2, space="PSUM") as ltpsp,
            ):
                xv = x_d.rearrange("(a p) c -> p a c", p=128)  # [128, NJ, D]
                # pass 1: DVE fp16 cast, fp16 PE transposes (1 cyc/row),
                # single ACT eviction into x16t
                for j4 in range(NJ // 4):
                    xj = xload.tile([128, 4, D], F32)
                    nc.sync.dma_start(xj[:], xv[:, j4 * 4:(j4 + 1) * 4, :])
                    if j4 < T:
                        # stream the weights in on the scalar HWDGE queue
                        i = j4
                        nc.scalar.dma_start(
                            w16[:, i, :, :],
                            w16_d[i].rearrange("(a p) c -> p a c", p=128))
                    x16j = xload.tile([128, 4, D], F16, tag="x16j", bufs=3,
                                      name="x16j")
                    nc.vector.tensor_copy(
                        x16j[:].rearrange("p a c -> p (a c)"),
                        xj[:].rearrange("p a c -> p (a c)"))
                    for jj in range(4):
                        j = j4 * 4 + jj
                        tp = xtps.tile([128, D], F16, name="tp")
                        for k in range(NK):
                            nc.tensor.transpose(tp[:, k * 128:(k + 1) * 128],
                                                x16j[:, jj, k * 128:(k + 1) * 128],
                                                ident16[:])
                        nc.scalar.activation(
                            x16t[:, :, j * 128:(j + 1) * 128],
                            tp[:].rearrange("p (k m) -> p k m", k=NK), Act.Copy)
                # pass 2: logits as L^T slices: lhsT=U16 (tiny LDW), f16 moving
                lt = ltsbp.tile([T, M], F32)
                for s in range(NS_L):
                    ltp = ltpsp.tile([T, SLL], F32)
                    for k in range(NK):
                        nc.tensor.matmul(
                            ltp[:], usb16[:, k, :],
                            x16t[:, k, s * SLL:(s + 1) * SLL],
                            start=(k == 0), stop=(k == NK - 1))
                    # evict with the per-step bias c fused in
                    nc.vector.tensor_scalar(lt[:, s * SLL:(s + 1) * SLL],
                                            ltp[:], ct[:], None, Alu.add)
                if fast0:
                    prt0 = small.tile([1, M], F16, name="prt0", tag="prt0")
                    nc.scalar.activation(prt0[:], lt[0:1, :], Act.Sigmoid)
                # pass 3: transpose L^T back to natural [128, NJ, T]
                for j in range(NJ):
                    lp = lpsp.tile([128, T], F32)
                    nc.tensor.transpose(lp[:], lt[:, j * 128:(j + 1) * 128],
                                        ident[0:T, 0:T])
                    nc.vector.tensor_copy(l32[:, j, :], lp[:])

            # ---- phase B: halting recurrence ([128, NJ] vectors), with the
            # per-step pack -> DRAM round trip -> broadcast chain pipelined
            # per i so phase D's step-i matmuls can start as soon as pb_i is
            # ready (PSUM accumulation groups tolerate gaps between steps).
            import contextlib
            _dstack = contextlib.ExitStack()
            packps = _dstack.enter_context(
                tc.tile_pool(name="packps", bufs=1, space="PSUM"))
            pbp = _dstack.enter_context(tc.tile_pool(name="pb", bufs=1))
            xsp = _dstack.enter_context(tc.tile_pool(name="xs", bufs=2))
            opsp = _dstack.enter_context(
                tc.tile_pool(name="ops", bufs=6, space="PSUM"))
            oevp = _dstack.enter_context(tc.tile_pool(name="oev", bufs=3))

            nc.vector.memset(hp[:], 0.0)
            nc.vector.memset(rem[:], 1.0)
            nc.vector.memset(nup[:], 0.0)
            p_t = small.tile([128, NJ], F32, name="rec_p", tag="rec_p")
            still = small.tile([128, NJ], F32, name="rec_still", tag="rec_still")
            ps = small.tile([128, NJ], F32, name="rec_ps", tag="rec_ps")
            hpp = small.tile([128, NJ], F32, name="rec_hpp", tag="rec_hpp")
            nh = small.tile([128, NJ], F32, name="rec_nh", tag="rec_nh")
            dd = small.tile([128, NJ], F32, name="rec_dd", tag="rec_dd")
            scratch = dramp.tile([T * NJ, 128], F16)
            pb_all = []
            prt_all = []
            for i in range(T):
                nc.scalar.activation(p_t[:], l32[:, :, i], Act.Sigmoid)
                if i == 0:
                    # step 0: hp=0 so still=1, rem=1 -> peff = p + nh*(1-p)
                    nc.vector.tensor_scalar(nh[:], p_t[:], float(TH), None,
                                            Alu.is_ge)
                    nc.vector.tensor_scalar(dd[:], p_t[:], -1.0, 1.0,
                                            Alu.mult, Alu.add)
                    nc.vector.tensor_tensor(dd[:], nh[:], dd[:], Alu.mult)
                    nc.vector.tensor_tensor(peff[:, i, :], p_t[:], dd[:],
                                            Alu.add)
                else:
                    nc.vector.tensor_scalar(still[:], hp[:], float(TH), None,
                                            Alu.is_lt)
                    nc.vector.tensor_tensor(ps[:], p_t[:], still[:], Alu.mult)
                    nc.vector.tensor_tensor(hpp[:], hp[:], ps[:], Alu.add)
                    nc.vector.tensor_scalar(nh[:], hpp[:], float(TH), None,
                                            Alu.is_ge)
                    # peff = ps + nh * (rem - ps)
                    nc.vector.tensor_tensor(dd[:], rem[:], ps[:], Alu.subtract)
                    nc.vector.tensor_tensor(dd[:], nh[:], dd[:], Alu.mult)
                    nc.vector.tensor_tensor(peff[:, i, :], ps[:], dd[:],
                                            Alu.add)
                # pack row i: transpose [128, NJ] -> [NJ, 128], cast fp16,
                # DRAM round trip to a [1, M] row, broadcast to [128, M].
                # Step 0 uses the precomputed sigmoid row directly.
                if i == 0 and fast0:
                    prt = prt0
                else:
                    pps = packps.tile([NJ, 128], F32)
                    nc.tensor.transpose(pps[:], peff[:, i, :], ident[:])
                    p16 = small.tile([NJ, 128], F16, tag="p16", bufs=2,
                                     name="p16")
                    nc.vector.tensor_copy(p16[:], pps[:])
                    nc.sync.dma_start(scratch[i * NJ:(i + 1) * NJ, :], p16[:])
                    prt = pbp.tile([1, M], F16, tag="prt", bufs=2, name="prt")
                    nc.sync.dma_start(
                        prt[:], scratch[i * NJ:(i + 1) * NJ, :]
                        .rearrange("a b -> (a b)").unsqueeze(0))
                pb = pbp.tile([128, M], F16, tag=f"pb{i}", name=f"pb{i}")
                # hybrid broadcast: gpsimd fills 32 partitions, two DMA
                # doublings fill the rest (DMA engines are idle here)
                nc.gpsimd.partition_broadcast(pb[0:32, :], prt[:])
                nc.scalar.dma_start(pb[32:64, :], pb[0:32, :])
                nc.scalar.dma_start(pb[64:128, :], pb[0:64, :])
                pb_all.append(pb)
                prt_all.append(prt)
                # state updates for the next step (off the pb critical path)
                if i == 0:
                    nc.vector.tensor_copy(hp[:], peff[:, i, :])
                    nc.vector.tensor_scalar(rem[:], peff[:, i, :], -1.0, 1.0,
                                            Alu.mult, Alu.add)
                    nc.vector.memset(nup[:], 1.0)
                else:
                    nc.vector.tensor_tensor(hp[:], hp[:], peff[:, i, :],
                                            Alu.add)
                    nc.vector.tensor_tensor(rem[:], rem[:], peff[:, i, :],
                                            Alu.subtract)
                    nc.vector.tensor_tensor(nup[:], nup[:], still[:], Alu.add)

            # ---- phase D: per m-slice, step-interleaved matmul
            # accumulation; per-tile step counts tij bound the work
            NS = 8                    # m-slices of 512
            SL = M // NS              # 512
            JPS = SL // 128           # m-tiles per slice = 4
            for s in range(NS):
                timax = max(tij[s * JPS:(s + 1) * JPS])
                ops_t = []
                for j2 in range(JPS):
                    ops = opsp.tile([128, D], F32, name=f"ops{j2}",
                                    tag=f"ops{j2 % 2}", bufs=4 - (j2 % 2))
                    ops_t.append(ops)
                for i in range(timax):
                    xs = xsp.tile([128, NK, SL], F16, tag=f"xs{i}",
                                  name=f"xs{i}")
                    nc.vector.tensor_tensor(
                        xs[:], x16t[:, :, s * SL:(s + 1) * SL],
                        pb_all[i][:, s * SL:(s + 1) * SL].unsqueeze(1)
                        .to_broadcast([128, NK, SL]), Alu.mult)
                    for j2 in range(JPS):
                        j = s * JPS + j2
                        ti = tij[j]
                        if i >= ti:
                            continue
                        nmm = ti * NK + (ti if use_dterm else 0)
                        n0 = i * (NK + (1 if use_dterm else 0))
                        for k in range(NK):
                            nc.tensor.matmul(
                                ops_t[j2][:],
                                xs[:, k, j2 * 128:(j2 + 1) * 128],
                                w16[:, i, k, :],
                                start=(n0 + k == 0),
                                stop=(n0 + k == nmm - 1))
                        if use_dterm:
                            nc.tensor.matmul(
                                ops_t[j2][:],
                                pb_all[i][0:1, j * 128:(j + 1) * 128],
                                dm16[0:1, i * D:(i + 1) * D],
                                start=False, stop=(n0 + NK == nmm - 1))
                for j2 in range(JPS):
                    j = s * JPS + j2
                    oev = oevp.tile([128, D], F32)
                    nc.scalar.activation(oev[:], ops_t[j2][:], Act.Copy)
                    nc.sync.dma_start(out_d[j * 128:(j + 1) * 128, :], oev[:])

            _dstack.close()
            nc.sync.dma_start(hp_d[:], hp[:])
            nc.sync.dma_start(rem_d[:], rem[:])
            nc.sync.dma_start(nup_d[:], nup[:])

    nc.compile()
    return nc


def _host_prep(x, W_halt, b_halt, W_layer, b_layer):
    """Derive step-linearized weights and the per-tile live-step counts.

    All output math runs on device; the host only computes the halting
    schedule (from the same logits the device evaluates) to skip matmul
    groups that are provably all-zero.  A borderline misprediction only
    costs a ~(1-THRESHOLD) weighted term, which is negligible.
    """
    Wl = np.asarray(W_layer, np.float64)
    Wh = np.asarray(W_halt, np.float64)
    bl = np.asarray(b_layer, np.float64)
    bh = np.asarray(b_halt, np.float64)

    U = np.zeros((D, MAX_STEPS), np.float64)
    c = np.zeros((1, MAX_STEPS), np.float64)
    Wpow = np.zeros((MAX_STEPS, D, D), np.float64)   # W^{i+1}
    Dmat = np.zeros((MAX_STEPS, D), np.float64)      # d_{i+1}
    Wi = np.eye(D)                                   # W^i
    di = np.zeros(D)                                 # d_i
    for i in range(MAX_STEPS):
        U[:, i] = (Wi @ Wh)[:, 0]
        c[0, i] = float(di @ Wh[:, 0] + bh[0])
        Wi = Wi @ Wl
        di = di @ Wl + bl
        Wpow[i] = Wi
        Dmat[i] = di

    # halting schedule (float64; decisions match the device fp32 path
    # except for benign near-threshold ties)
    logits = x.reshape(-1, D).astype(np.float64) @ U + c  # [B*S, 10]
    p_all = 1.0 / (1.0 + np.exp(-logits))
    hp = np.zeros(B * S)
    live = np.zeros(B * S, np.int64)
    active = np.ones(B * S, bool)
    for i in range(MAX_STEPS):
        live[active] = i + 1
        nhl = active & (hp + p_all[:, i] >= float(TH))
        hp = np.where(active, hp + p_all[:, i], hp)
        active = active & ~nhl
        if not active.any():
            break
    live[active] = MAX_STEPS
    # per-tile union across cores: tile j covers rows j*128..(j+1)*128 of
    # every core's M-row shard
    lv = live.reshape(NCORES, NJ, 128)
    tij = tuple(int(lv[:, j, :].max()) for j in range(NJ))
    T = max(tij)
    use_dterm = bool(np.any(Dmat[:T] != 0.0))
    # step-0 fast path: broadcast sigmoid(logit_0) directly as peff_0.
    # For the few positions with p_0 >= TH the true peff_0 is rem=1, an
    # error bounded by (1-TH)*|state| each -- same magnitude as a benign
    # near-threshold tie. Only allow it when that count is tiny.
    fast0 = bool(int((p_all[:, 0] >= float(TH) - 1e-4).sum()) <= 8)
    return (U[:, :T].astype(np.float32).copy(),
            c[:, :T].astype(np.float32).copy(),
            np.ascontiguousarray(Wpow[:T].astype(np.float16)),
            np.ascontiguousarray(Dmat[:T].astype(np.float16)),
            use_dterm, tij, fast0)


def kernel(x, W_halt, b_halt, W_layer, b_layer):
    global LAST_RESULTS
    x = np.ascontiguousarray(np.asarray(x, np.float32))
    assert x.shape == (B, S, D)

    U, c, Wpow16, Dmat16, use_dterm, tij, fast0 = _host_prep(
        x, W_halt, b_halt, W_layer, b_layer)
    key = (use_dterm, tij, fast0)
    if key not in _CACHE:
        _CACHE[key] = _build(use_dterm, tij, fast0)
    nc = _CACHE[key]

    ident = np.eye(128, dtype=np.float32)
    shards = x.reshape(NCORES, M, D)
    in_maps = []
    for cidx in range(NCORES):
        in_maps.append({
            "x": shards[cidx], "ident": ident, "u": U, "c": c,
            "w16": Wpow16, "dm16": Dmat16,
        })
    res = bass_utils.run_bass_kernel_spmd(
        nc, in_maps, core_ids=list(range(NCORES)), trace=TRACE)
    LAST_RESULTS = res

    out = np.empty((NCORES, M, D), np.float32)
    hp_all = np.empty((NCORES, 128, NJ), np.float64)
    rem_all = np.empty_like(hp_all)
    nup_all = np.empty_like(hp_all)
    for cidx in range(NCORES):
        r = res.results[cidx]
        out[cidx] = r["out"]
        hp_all[cidx] = r["hp"]
        rem_all[cidx] = r["rem"]
        nup_all[cidx] = r["nup"]

    ponder_cost = np.float32((nup_all + rem_all).mean())
    avg_steps = np.float32(nup_all.mean())
    hp_mean = np.float32(hp_all.mean())
    return (out.reshape(B, S, D), ponder_cost, avg_steps, hp_mean)
